# revision 34
# baseline (speedup 1.0000x reference)
"""Trainium2 Bass kernel for nn_Classifier_48223892799748 (retrieval_knn).

Computes sim = (D + enc_pm @ cent_pm.T) / 2 where
  enc_pm = sign((samples - 0.5) @ weight.T)  in {+1,-1}
  cent_pm = centroids mapped {0,1} -> {-1,+1}

Sharding: data-parallel over the batch dim (8192 -> 1024 rows per core,
8 cores). weight / centroids replicated.

Device layout: everything is computed transposed ([D, B] / [C, B]) so that
the sign-encoded matmul-1 output tile [128 d, 512 b] feeds matmul-2
directly as the moving operand (contraction over d) with no on-device
transpose.

Matmul-1 runs in fp8e4m3 DoubleRow (2x bf16 MAC rate; the stream floor
is 512 cols @2.4GHz = ~216ns per [256k x 128d x 512b] matmul): samples
are quantized to e4m3 on the host, weights (+/-1) are exact in fp8.
This flips ~85 of the 10000 sign bits per batch row (quantization noise
~0.64 vs proj sigma ~36), moving each output count by at most ~47 of an
allowed ~105 (rel gate 2e-2 at scale 5260): measured rel err 0.0089,
identical on host emulation and hardware. Matmul-2 is exact.

Sign encodings are split across engines so neither gates the PE stream:
  b-chunk 0: ScalarE Sign        -> enc in {+1,-1}; out = 0.5*ps2 + D/2
  b-chunk 1: DVE    is_gt(0)     -> enc in {0,1};   out = ps2 + (D-colsum)/2
(the {0,1} encoding needs the per-centroid colsum correction, shipped as
a tiny [C,1] bias vector computed on host).

Startup choreography (worth ~14us vs naive):
  - WARM_N dummy N=512 DR matmuls on a memset tile at the head of the PE
    queue: the HAM clock gate only reaches 8/8 (2.4 GHz) after ~3.4us of
    GAPLESS streaming (N=128 dummies at ~84% busy never latch it), and
    the DMA system cannot feed real work before ~12us anyway (fixed ~7us
    runtime preamble + ~650ns/trigger sync-engine serialization + ramp).
  - dt=0's weight tile is DMA'd as the second trigger into a const tile;
    the 1.2MB centroid load is deferred to dt=24 (needed ~150us in).
  - single DMA per 128KB weight tile (transfers stripe across all 16
    queues; more triggers only serialize the sync engine).

Legacy modes (f32r / bf16_hilo / fp16 / bf16 matmul-1) remain selectable
via MM1_MODE for A/B testing.
"""

import sys

if "/opt/trn_rl_repo" not in sys.path:
    sys.path.insert(0, "/opt/trn_rl_repo")

import ml_dtypes
import numpy as np

import concourse.bass as bass
import concourse.mybir as mybir
import concourse.tile as tile
from concourse import bacc
from concourse.bass_utils import run_bass_kernel_spmd

# The container's `antenv` package is a stub without `axon_hooks`; if tracing
# is ever requested (BASS_TRACE=1), run_bass_kernel_spmd imports it and would
# crash. Provide a stub module (hook=None -> tracing skipped gracefully)
# unless something (e.g. a test harness) registered a real one already.
try:  # pragma: no cover
    import antenv.axon_hooks  # noqa: F401
except ImportError:
    import types as _types

    import antenv as _antenv

    _hooks = _types.ModuleType("antenv.axon_hooks")
    _hook_store = {"h": None}
    _hooks.set_axon_ntff_profile_hook = lambda h: _hook_store.__setitem__("h", h)
    _hooks.get_axon_ntff_profile_hook = lambda: _hook_store["h"]
    sys.modules["antenv.axon_hooks"] = _hooks
    _antenv.axon_hooks = _hooks

BF16 = ml_dtypes.bfloat16
FP8 = ml_dtypes.float8_e4m3

B, IN_F, D, C = 8192, 1024, 10000, 100
N_CORES = 8
B_SH = B // N_CORES          # 1024 batch rows per core
KC = IN_F // 128             # 8 contraction chunks for matmul 1
KCP = KC // 2                # 4 DoubleRow contraction pairs
DT = (D + 127) // 128        # 79 d-tiles
D_PAD = DT * 128             # 10112
NB = B_SH // 512             # 2 psum-width chunks of the local batch
CENTER = 0.5

# matmul-1 mode: "fp8_dr" | "f32r" | "bf16_hilo" | "fp16" | "bf16"
import os as _os
MM1_MODE = _os.environ.get("MM1_MODE", "fp8_dr")
USE_F32R = MM1_MODE == "f32r"
USE_FP8DR = MM1_MODE == "fp8_dr"
# split sign-encoding across ScalarE+DVE (fp8_dr path only)
SIGN_SPLIT = _os.environ.get("SIGN_SPLIT", "1") == "1"
# dummy warm-up matmuls at the head of the PE queue: keep the PE busy
# through the ~5us DMA spin-up + ramp so the HAM clock gate reaches
# 8/8 (2.4 GHz) before the real stream starts (else the first ~26us
# run at 1.2 GHz)
WARM_N = int(_os.environ.get("WARM_N", "11"))
# how many d-tiles into the stream to defer the cent/bv DMA (keeps the
# 1.2 MB centroid load out of the startup window; MM2 needs it ~80us in)
CENT_AT = int(_os.environ.get("CENT_AT", "24"))
NPAIR = (DT + 1) // 2        # 40 d-tile pairs for DoubleRow matmul-2
D_PAD2 = NPAIR * 256         # 10240
C_PAD = 112                  # DoubleRow weight AP needs byte-step %16 == 0

# Stash of the last BassKernelResults (exec_time_ns etc.) for test harnesses.
LAST_RUN = None
_NC_CACHE = None


def _build_nc():
    nc = bacc.Bacc("TRN2", target_bir_lowering=False)
    f32 = mybir.dt.float32
    f32r = mybir.dt.float32r
    bf16 = mybir.dt.bfloat16
    fp8 = mybir.dt.float8e4
    SIGN = mybir.ActivationFunctionType.Sign
    COPY = mybir.ActivationFunctionType.Copy
    IDENT = mybir.ActivationFunctionType.Identity
    DR = mybir.MatmulPerfMode.DoubleRow

    # DRAM I/O (per-core shard layouts, see host prep in kernel()):
    #   fp8_dr path:
    #     sq: [128 ki, KCP, 2, B_SH] fp8   e4m3(samples-0.5).T, k = j*256+jo*128+ki
    #     wt: [DT, 128 ki, KCP, 2, 128 d] fp8  weight.T DR tiles (+/-1)
    #     bv: [C, 1] f32                   (D - colsum(cent_pm))/2 bias
    #   f32r path:
    #     sf: [128 k_in, KC, B_SH] f32     (samples-0.5).T
    #     wt: [DT, 128 k_in, KC, 128 d_in] f32r  weight.T tiles (+/-1)
    #   ct:  [128 d_in, NPAIR, 2, C_PAD] fp8  centroids.T DR tiles (+/-1)
    #   out: [C, B_SH] f32                 sim.T shard
    fp16 = mybir.dt.float16
    if USE_FP8DR:
        sq_d = nc.dram_tensor("sq", [128, KCP, 2, B_SH], fp8, kind="ExternalInput")
        wt_d = nc.dram_tensor("wt", [DT, 128, KCP, 2, 128], fp8, kind="ExternalInput")
        bv_d = nc.dram_tensor("bv", [C, 1], f32, kind="ExternalInput")
        lp = fp8
    else:
        lp = {"f32r": f32r, "bf16_hilo": bf16, "fp16": fp16, "bf16": bf16}[MM1_MODE]
        if USE_F32R:
            sf_d = nc.dram_tensor("sf", [128, KC, B_SH], f32, kind="ExternalInput")
            wt_d = nc.dram_tensor("wt", [DT, 128, KC, 128], f32r, kind="ExternalInput")
        elif MM1_MODE == "bf16_hilo":
            sh_d = nc.dram_tensor("sh", [128, KC, B_SH], bf16, kind="ExternalInput")
            sl_d = nc.dram_tensor("sl", [128, KC, B_SH], bf16, kind="ExternalInput")
            wt_d = nc.dram_tensor("wt", [DT, 128, KC, 128], bf16, kind="ExternalInput")
        else:
            sh_d = nc.dram_tensor("sh", [128, KC, B_SH], lp, kind="ExternalInput")
            wt_d = nc.dram_tensor("wt", [DT, 128, KC, 128], lp, kind="ExternalInput")
    ct_d = nc.dram_tensor("ct", [128, NPAIR, 2, C_PAD], fp8, kind="ExternalInput")
    out_d = nc.dram_tensor("out", [C, B_SH], f32, kind="ExternalOutput")

    w_dt = lp

    with tile.TileContext(nc) as tc:
        with (
            tc.tile_pool(name="const", bufs=1) as const_pool,
            tc.tile_pool(name="wts", bufs=8) as w_pool,
            tc.tile_pool(name="outp", bufs=1) as out_pool,
            tc.tile_pool(name="ps1", bufs=3, space=bass.MemorySpace.PSUM) as ps1_pool,
            tc.tile_pool(name="ps2", bufs=1, space=bass.MemorySpace.PSUM) as ps2_pool,
        ):
            preamble_rest = None
            if USE_FP8DR and WARM_N > 0:
                # N=512 so the dummy stream is GAPLESS: the HAM activity
                # window only latches warm (K=8/8) under back-to-back
                # saturated streaming; N=128 dummies (~84% busy) never do
                warm = const_pool.tile([128, 2, 512], fp8)
                # memset via uint32 bitcast: 4x fewer stores than byte-wise,
                # and gpsimd needs no activation-table load (ScalarE's COPY
                # does, costing an extra 1.3us before the first dummy)
                nc.gpsimd.memset(warm[:].bitcast(mybir.dt.uint32), 0)
                warm_ps = ps1_pool.tile(
                    [128, 512], mybir.dt.float32, tag="ps1_0", name="warm_ps"
                )
                DRm = mybir.MatmulPerfMode.DoubleRow
                for _ in range(WARM_N):
                    nc.tensor.matmul(
                        warm_ps[:],
                        warm[:, :, :128],
                        warm[:],
                        start=True,
                        stop=True,
                        perf_mode=DRm,
                    )
            if USE_FP8DR:
                s_q = const_pool.tile([128, KCP, 2, B_SH], fp8)
                # startup triggers split across the two HWDGE queues (sync +
                # scalar): trigger instructions serialize at ~650ns each per
                # queue, and the DMA ramp is trigger-feed-bound — two queues
                # double the early delivery rate. ScalarE is free until its
                # first Sign at ~13us.
                # dt=0's whole weight tile lives in the const pool and is
                # DMA'd at the head of the scalar trigger queue, so the
                # w_pool stream (dt>=1) never gates the start
                w_early = const_pool.tile([128, KCP, 2, 128], fp8)
                nc.scalar.dma_start(w_early[:], wt_d[0])
                nc.sync.dma_start(
                    s_q[:, 0, :, bass.ts(0, 512)], sq_d[:, 0, :, bass.ts(0, 512)]
                )
                nc.scalar.dma_start(
                    s_q[:, 0, :, bass.ts(1, 512)], sq_d[:, 0, :, bass.ts(1, 512)]
                )

                def preamble_rest():
                    nc.sync.dma_start(s_q[:, 1, :, :], sq_d[:, 1, :, :])
                    nc.scalar.dma_start(s_q[:, 2, :, :], sq_d[:, 2, :, :])
                    nc.sync.dma_start(s_q[:, 3, :, :], sq_d[:, 3, :, :])

                bv_t = const_pool.tile([C, 1], f32)
            elif USE_F32R:
                s_f = const_pool.tile([128, KC, B_SH], f32)
                s_r = const_pool.tile([128, KC, B_SH], f32r)
                # per-kc loads + f32->f32r rounding casts (DVE is otherwise
                # idle); split so PE can start after the first chunk.
                for b in range(NB):
                    nc.sync.dma_start(
                        s_f[:, 0, bass.ts(b, 512)], sf_d[:, 0, bass.ts(b, 512)]
                    )
                    nc.vector.tensor_copy(
                        s_r[:, 0, bass.ts(b, 512)], s_f[:, 0, bass.ts(b, 512)]
                    )

                def preamble_rest():
                    for kc in range(1, KC):
                        nc.sync.dma_start(s_f[:, kc, :], sf_d[:, kc, :])
                        nc.vector.tensor_copy(s_r[:, kc, :], s_f[:, kc, :])

                w00 = const_pool.tile([128, 128], f32r)
                nc.sync.dma_start(w00[:], wt_d[0, :, 0, :])
                s_streams = [s_r]
            elif MM1_MODE == "bf16_hilo":
                s_hi = const_pool.tile([128, KC, B_SH], bf16)
                s_lo = const_pool.tile([128, KC, B_SH], bf16)
                for kc in range(KC):
                    nc.sync.dma_start(s_hi[:, kc, :], sh_d[:, kc, :])
                    nc.sync.dma_start(s_lo[:, kc, :], sl_d[:, kc, :])
                s_streams = [s_hi, s_lo]
            else:
                s_hi = const_pool.tile([128, KC, B_SH], lp)
                for kc in range(KC):
                    nc.sync.dma_start(s_hi[:, kc, :], sh_d[:, kc, :])
                s_streams = [s_hi]

            cent = const_pool.tile([128, NPAIR, 2, C_PAD], fp8)
            # all sign-encodings buffered on-chip; matmul-2 runs as one
            # uniform fp8-DoubleRow block after the matmul-1 stream ends
            enc_all = const_pool.tile([128, NPAIR, 2, B_SH], fp8)
            # phantom j=1 half of the final pair (dt=79 doesn't exist):
            # zero it so 0-weight x garbage(NaN) can't poison the PSUM
            nc.gpsimd.memset(enc_all[:, NPAIR - 1, 1, :], 0.0)

            ps2 = [
                ps2_pool.tile([C_PAD, 512], mybir.dt.float32, tag=f"ps2_{b}", name=f"ps2_{b}")
                for b in range(NB)
            ]

            for dt in range(DT):
                if USE_FP8DR:
                    if dt == 0 and preamble_rest is not None:
                        # remaining sample chunks BEFORE the weight stream:
                        # the sync engine issues triggers serially and all of
                        # s_q is needed within dt=0
                        preamble_rest()
                        preamble_rest = None
                        w = None
                    else:
                        w = w_pool.tile(
                            [128, KCP, 2, 128], fp8, tag="w", name=f"w_{dt}"
                        )
                        # single DMA per tile: the sync engine serializes
                        # trigger instructions at ~650ns each, and transfers
                        # are striped across all 16 queues anyway
                        nc.sync.dma_start(w[:], wt_d[dt])
                else:
                    w = w_pool.tile([128, KC, 128], w_dt, tag="w", name=f"w_{dt}")
                    nc.sync.dma_start(w[:, : KC // 2, :], wt_d[dt, :, : KC // 2, :])
                    nc.sync.dma_start(w[:, KC // 2 :, :], wt_d[dt, :, KC // 2 :, :])
                if dt == 0:
                    # deferred preamble: remaining sample chunks
                    if preamble_rest is not None:
                        preamble_rest()
                    if not USE_FP8DR:
                        nc.sync.dma_start(cent[:], ct_d[:])
                if dt == (CENT_AT if USE_FP8DR else DT):
                    # centroids deferred out of the startup DMA window
                    nc.sync.dma_start(cent[:], ct_d[:])
                    nc.sync.dma_start(bv_t[:], bv_d[:])
                ps1 = [
                    ps1_pool.tile(
                        [128, 512], mybir.dt.float32, tag=f"ps1_{b}", name=f"ps1_{dt}_{b}"
                    )
                    for b in range(NB)
                ]
                if USE_FP8DR:
                    for j in range(KCP):
                        w_src = w_early[:, j, :, :] if dt == 0 else w[:, j, :, :]
                        for b in range(NB):
                            nc.tensor.matmul(
                                ps1[b][:],
                                w_src,
                                s_q[:, j, :, bass.ts(b, 512)],
                                start=(j == 0),
                                stop=(j == KCP - 1),
                                perf_mode=DR,
                            )
                else:
                    n_acc = len(s_streams) * KC
                    acc = 0
                    for kc in range(KC):
                        w_src = w00 if (USE_F32R and dt == 0 and kc == 0) else w[:, kc, :]
                        for s_t in s_streams:
                            for b in range(NB):
                                nc.tensor.matmul(
                                    ps1[b][:],
                                    w_src,
                                    s_t[:, kc, bass.ts(b, 512)],
                                    start=(acc == 0),
                                    stop=(acc == n_acc - 1),
                                )
                            acc += 1
                for b in range(NB):
                    dst = enc_all[:, dt // 2, dt % 2, bass.ts(b, 512)]
                    if USE_FP8DR and SIGN_SPLIT and b == 1:
                        # DVE: enc in {0,1}; corrected via bv in the output
                        nc.vector.tensor_scalar(
                            dst, ps1[b][:], 0.0, None, mybir.AluOpType.is_gt
                        )
                    else:
                        nc.scalar.activation(dst, ps1[b][:], SIGN)

            # b-major: ps2[0] completes ~8.6us before ps2[1], so b=0's
            # output copy + DMA overlap b=1's matmul block
            for b in range(NB):
                for t in range(NPAIR):
                    nc.tensor.matmul(
                        ps2[b][:],
                        cent[:, t, :, :],
                        enc_all[:, t, :, bass.ts(b, 512)],
                        start=(t == 0),
                        stop=(t == NPAIR - 1),
                        perf_mode=DR,
                    )

            for b in range(NB):
                ob = out_pool.tile([C, 512], mybir.dt.float32, tag=f"ob_{b}", name=f"ob_{b}")
                if USE_FP8DR and SIGN_SPLIT and b == 1:
                    # enc in {0,1}: sim = ps2 + (D - colsum)/2, on DVE so both
                    # output chunks post-process in parallel
                    nc.vector.tensor_scalar(
                        ob[:], ps2[b][:C, :], bv_t[:], None, mybir.AluOpType.add
                    )
                else:
                    nc.scalar.activation(
                        ob[:], ps2[b][:C, :], COPY, bias=D / 2.0, scale=0.5
                    )
                nc.sync.dma_start(out_d[:, bass.ts(b, 512)], ob[:])

    nc.compile()
    return nc


def _get_nc():
    global _NC_CACHE
    if _NC_CACHE is None:
        _NC_CACHE = _build_nc()
    return _NC_CACHE


def kernel(samples, weight, centroids):
    global LAST_RUN
    samples = np.asarray(samples, dtype=np.float32)
    weight = np.asarray(weight, dtype=np.float32)
    centroids = np.asarray(centroids)

    # ---- host-side marshalling (layout + dtype only) ----
    # centered samples, transposed to [IN_F, B]
    scT = (samples - np.float32(CENTER)).T

    # DoubleRow centroid tiles: ct[d_in, t, j, c] = cent_pm[c, t*256+j*128+d_in]
    cent_pm = np.where(centroids, np.float32(1.0), np.float32(-1.0))
    cpad = np.zeros((D_PAD2, C_PAD), dtype=np.float32)
    cpad[:D, :C] = cent_pm.T
    ct = np.ascontiguousarray(
        cpad.reshape(NPAIR, 2, 128, C_PAD).transpose(2, 0, 1, 3).astype(FP8)
    )

    if USE_FP8DR:
        sq8 = scT.astype(FP8)

        def s_core(c):
            # [IN_F, B_SH] -> [128 ki, KCP, 2, B_SH], k = j*256 + jo*128 + ki
            blk = sq8[:, c * B_SH : (c + 1) * B_SH]
            return np.ascontiguousarray(
                blk.reshape(KCP, 2, 128, B_SH).transpose(2, 0, 1, 3)
            )

        # weight.T DR tiles: wt[dt, ki, j, jo, d_in] = w[dt*128+d_in, j*256+jo*128+ki]
        wpad = np.zeros((D_PAD, IN_F), dtype=FP8)
        wpad[:D] = weight.astype(FP8)  # +/-1, exact in fp8
        wt = np.ascontiguousarray(
            wpad.reshape(DT, 128, KCP, 2, 128).transpose(0, 4, 2, 3, 1)
        )
        # bias vector for the {0,1}-encoded chunk: (D - colsum(cent_pm))/2
        bv = ((np.float32(D) - cent_pm.sum(axis=1, dtype=np.float32)) * 0.5).astype(
            np.float32
        )[:, None]
        in_maps = [
            {"sq": s_core(c), "wt": wt, "ct": ct, "bv": bv} for c in range(N_CORES)
        ]
    else:
        FP16 = np.float16
        w_np = {"f32r": np.float32, "bf16_hilo": BF16, "fp16": FP16, "bf16": BF16}[
            MM1_MODE
        ]

        def s_core_legacy(a, c):
            # [IN_F, B_SH] -> [128 k_in, KC, B_SH]
            blk = a[:, c * B_SH : (c + 1) * B_SH]
            return np.ascontiguousarray(blk.reshape(KC, 128, B_SH).transpose(1, 0, 2))

        # weight.T tiles: wt[dt, k_in, kc, d_in] = weight[dt*128+d_in, kc*128+k_in]
        wpad = np.zeros((D_PAD, IN_F), dtype=w_np)
        wpad[:D] = weight.astype(w_np)  # +/-1, exact in bf16/f32r
        wt = np.ascontiguousarray(wpad.reshape(DT, 128, KC, 128).transpose(0, 3, 2, 1))

        if USE_F32R:
            in_maps = [
                {"sf": s_core_legacy(scT, c), "wt": wt, "ct": ct}
                for c in range(N_CORES)
            ]
        elif MM1_MODE == "bf16_hilo":
            s_hi = scT.astype(BF16)
            s_lo = (scT - s_hi.astype(np.float32)).astype(BF16)
            in_maps = [
                {
                    "sh": s_core_legacy(s_hi, c),
                    "sl": s_core_legacy(s_lo, c),
                    "wt": wt,
                    "ct": ct,
                }
                for c in range(N_CORES)
            ]
        else:
            s_hi = scT.astype(w_np)
            in_maps = [
                {"sh": s_core_legacy(s_hi, c), "wt": wt, "ct": ct}
                for c in range(N_CORES)
            ]

    nc = _get_nc()
    res = run_bass_kernel_spmd(nc, in_maps, core_ids=list(range(N_CORES)))
    LAST_RUN = res

    # gather: out[c] is sim.T for batch rows [c*B_SH, (c+1)*B_SH)
    return np.vstack(
        [np.asarray(res.results[c]["out"]).T for c in range(N_CORES)]
    ).astype(np.float32)
